# revision 29
# baseline (speedup 1.0000x reference)
"""Trainium2 kernel for BottomUpAttention (gnn_message_passing).

Math note: the reference applies softmax over a singleton axis
(``softmax(scores[:, None], axis=1)``), which is identically 1.0 for every
cell, so the attention branch (cell_keys / tissue_q / tanh / attn_w) cannot
affect the output.  The module reduces exactly to

    out = tissue_features + segment_sum(cell_features, cluster_assignments)

which is a memory-bound scatter-add over the cell features.

Strategy (8 NeuronCores, SPMD, no collectives):
  * Shard by *tissue*: each core owns 625 tissues, grouped into 5 blocks of
    125.  Tissues are greedily packed into blocks by descending cell count
    so every block has a near-equal number of cells (minimises padding).
  * Host argsorts cells by tissue id and packs each block's cells into
    128-row tiles, padded to a common tile count T_b so all cores run the
    identical SPMD program.
  * Cell rows are stored fp16 (sum error ~1e-3 abs vs output scale ~45 —
    far below the 2e-2 gate).  The stream is laid out partition-major in
    DMA groups of up to 32 tiles; each group chunk is [localid row | 32
    tiles of 256 features] per partition, so the per-cell segment ids ride
    inside the same large-descriptor stream (separate small-descriptor
    loads get starved by round-robin against 16KB descriptors).
  * Group sizes ramp 8,8,16,16,32,... so the first matmul can start ~1.5us
    after the first DMA instead of waiting for a full 2MB group.
  * On device, each 128-cell tile is reduced by one one-hot matmul into
    the block's [125, 256] fp32 PSUM accumulator: lhsT[i, j] =
    (localid[i] == j).  One-hots for 4 tiles at a time are built by a
    tensor_tensor(is_equal) against an iota row (generated on device),
    alternating between the DVE and GpSimd engines so neither is the
    bottleneck.
  * Cell-data DMA groups alternate between the sync and scalar HWDGE
    queues; the tissue-feature load is deferred behind the first groups.
  * After a block's tiles are accumulated, out = psum + tissue slice is a
    single DVE add; per-block output DMAs overlap with the next block.
    Outputs are [125, 5*256] per core; the host inverse-permutes rows into
    the final [5000, 256].
"""

import numpy as np

P = 128          # SBUF partitions / matmul contraction dim
NCORES = 8
BLK = 125        # tissues per block (PSUM partition rows, <=128)
G = 32           # 128-cell tiles per DMA group (32 -> ~2.1 MiB loads)

LAST_RESULTS = None  # BassKernelResults of the most recent kernel() call

_PROGRAM_CACHE = {}


def _group_schedule(NT):
    """Uniform groups, strictly alternating DMA queues (keeps the two
    HWDGE queues byte-balanced so neither systematically lags)."""
    sizes = []
    while sum(sizes) < NT:
        sizes.append(min(G, NT - sum(sizes)))
    starts = np.cumsum([0] + sizes[:-1]).tolist()
    return tuple(zip(starts, sizes))


def _build_program(NT, T_b, NBLK, DIM):
    import concourse.mybir as mybir
    import concourse.tile as tile
    from concourse import bacc

    f32 = mybir.dt.float32
    f16 = mybir.dt.float16
    DIMP = DIM + 1
    groups = _group_schedule(NT)

    nc = bacc.Bacc(
        "TRN2",
        target_bir_lowering=False,
        debug=False,
        enable_asserts=False,
        num_devices=NCORES,
    )
    x = nc.dram_tensor("x", [P, NT * DIMP], f16, kind="ExternalInput")
    iota = nc.dram_tensor("iota", [P, G * BLK], f16, kind="ExternalInput")
    tqp = nc.dram_tensor("tqp", [BLK, NBLK * DIM], f32, kind="ExternalInput")
    y = nc.dram_tensor("y", [BLK, NBLK * DIM], f32, kind="ExternalOutput")

    with tile.TileContext(nc) as tc:
        with (
            tc.tile_pool(name="const", bufs=1) as cpool,
            tc.tile_pool(name="data", bufs=8) as dpool,
            tc.tile_pool(name="oh", bufs=4) as ohpool,
            tc.tile_pool(name="psum", bufs=3, space="PSUM") as ppool,
        ):
            iota_sb = cpool.tile([P, G * BLK], f16)
            nc.sync.dma_start(out=iota_sb[:], in_=iota[:])
            tqp_sb = cpool.tile([BLK, NBLK * DIM], f32)
            out_sb = cpool.tile([BLK, NBLK * DIM], f32)

            # dummy matmuls on the iota tile: keeps the PE HAM busy during
            # the DMA head so real matmuls start at the warm 2.4 GHz clock
            warm = ppool.tile([BLK, DIM], f32, tag="warm")
            for wi in range(24):
                nc.tensor.matmul(
                    out=warm[:],
                    lhsT=iota_sb[:, DIM : DIM + BLK],
                    rhs=iota_sb[:, :DIM],
                    start=(wi == 0),
                    stop=(wi == 23),
                )

            pending = []  # completed blocks awaiting epilogue emission

            def _flush():
                while pending:
                    b, bps = pending.pop(0)
                    osl = out_sb[:, b * DIM : (b + 1) * DIM]
                    nc.vector.tensor_tensor(
                        out=osl,
                        in0=bps[:],
                        in1=tqp_sb[:, b * DIM : (b + 1) * DIM],
                        op=mybir.AluOpType.add,
                    )
                    yeng = nc.sync if (b % 2 == 1) else nc.scalar
                    yeng.dma_start(
                        out=y[:, b * DIM : (b + 1) * DIM], in_=osl
                    )

            ps = None
            for gi, (t0, gn) in enumerate(groups):
                dt_ = dpool.tile([P, G * DIMP], f16, tag="data")
                eng = nc.sync if (gi % 2 == 0) else nc.scalar
                eng.dma_start(
                    out=dt_[:, : gn * DIMP],
                    in_=x[:, t0 * DIMP : (t0 + gn) * DIMP],
                )
                if gi == 3:
                    # deferred: only needed by the first block's epilogue
                    nc.scalar.dma_start(out=tqp_sb[:], in_=tqp[:])
                _flush()
                ohg = ohpool.tile([P, G, BLK], f16, tag="oh")
                nc.vector.tensor_tensor(
                    out=ohg[:, :gn, :],
                    in0=iota_sb[:, : gn * BLK].rearrange(
                        "p (k c) -> p k c", k=gn
                    ),
                    in1=dt_[:, :gn]
                    .rearrange("p (k o) -> p k o", o=1)
                    .to_broadcast([P, gn, BLK]),
                    op=mybir.AluOpType.is_equal,
                )
                for j in range(gn):
                    gt = t0 + j
                    b, rel = divmod(gt, T_b)
                    if rel == 0:
                        ps = ppool.tile([BLK, DIM], f32, tag="ps")
                    nc.tensor.matmul(
                        out=ps[:],
                        lhsT=ohg[:, j, :],
                        rhs=dt_[:, gn + j * DIM : gn + (j + 1) * DIM],
                        start=(rel == 0),
                        stop=(rel == T_b - 1),
                    )
                    if rel == T_b - 1:
                        pending.append((b, ps))
            _flush()
    nc.compile()
    return nc


def kernel(
    cell_features,
    tissue_features,
    cluster_assignments,
    W_cell,
    b_cell,
    W_tissue,
    b_tissue,
    attn_w,
):
    global LAST_RESULTS
    from concourse.bass_utils import run_bass_kernel_spmd

    cells = np.asarray(cell_features, dtype=np.float32)
    tissue = np.asarray(tissue_features, dtype=np.float32)
    assign = np.asarray(cluster_assignments).astype(np.int64)

    n_cell, DIM = cells.shape
    n_tissue = tissue.shape[0]
    assert n_tissue % (NCORES * BLK) == 0, (n_tissue, NCORES, BLK)
    TPC = n_tissue // NCORES       # tissues per core
    NBLK = TPC // BLK              # blocks per core
    nblocks_g = NCORES * NBLK
    DIMP = DIM + 1

    hi = cells.astype(np.float16)

    # ---- host: balance tissues into blocks by cell count (less padding) ----
    tcounts = np.bincount(assign, minlength=n_tissue)
    t_order_desc = np.argsort(-tcounts, kind="stable")
    block_sum = np.zeros(nblocks_g, dtype=np.int64)
    block_fill = np.zeros(nblocks_g, dtype=np.int64)
    tissue2block = np.empty(n_tissue, dtype=np.int64)
    tissue2loc = np.empty(n_tissue, dtype=np.int64)
    import heapq

    heap = [(0, b) for b in range(nblocks_g)]
    heapq.heapify(heap)
    for t in t_order_desc:
        while True:
            s, b = heapq.heappop(heap)
            if block_fill[b] < BLK:
                break
        tissue2block[t] = b
        tissue2loc[t] = block_fill[b]
        block_fill[b] += 1
        block_sum[b] += tcounts[t]
        if block_fill[b] < BLK:
            heapq.heappush(heap, (block_sum[b], b))

    T_b = max(1, int(-(-block_sum.max() // P)))  # tiles per block (all cores)
    CAP = T_b * P
    NT = NBLK * T_b
    groups = _group_schedule(NT)

    # ---- host: sort cells by (block, position) and pack per core ----
    cell_block = tissue2block[assign]
    order = np.argsort(cell_block, kind="stable").astype(np.int64)
    sorted_block = cell_block[order]
    cuts = np.searchsorted(sorted_block, np.arange(nblocks_g + 1))
    loc_of_cell = tissue2loc[assign].astype(np.float16)

    # tissue rows permuted to (block, localid) layout
    tissue_rows = np.zeros((nblocks_g, BLK, DIM), dtype=np.float32)
    tissue_rows[tissue2block, tissue2loc] = tissue

    iota_np = np.ascontiguousarray(
        np.tile(np.arange(BLK, dtype=np.float16), (P, G))
    )

    in_maps = []
    for k in range(NCORES):
        pi = np.zeros(NBLK * CAP, dtype=np.int64)
        lo_ids = np.full(NBLK * CAP, float(BLK), dtype=np.float16)  # pad -> no hit
        for b in range(NBLK):
            i = k * NBLK + b
            seg = order[cuts[i] : cuts[i + 1]]
            pi[b * CAP : b * CAP + len(seg)] = seg
            lo_ids[b * CAP : b * CAP + len(seg)] = loc_of_cell[seg]
        piT = pi.reshape(NT, P)            # [NT, P]
        locT = lo_ids.reshape(NT, P).T     # [P, NT]
        xk = np.empty((P, NT * DIMP), dtype=np.float16)
        for t0, gn in groups:
            off = t0 * DIMP
            xk[:, off : off + gn] = locT[:, t0 : t0 + gn]
            blockdata = hi[piT[t0 : t0 + gn].T]          # [P, gn, DIM]
            xk[:, off + gn : off + gn * DIMP] = blockdata.reshape(P, gn * DIM)
        tqp = np.ascontiguousarray(
            tissue_rows[k * NBLK : (k + 1) * NBLK]
            .transpose(1, 0, 2)
            .reshape(BLK, NBLK * DIM)
        )
        in_maps.append({"x": xk, "iota": iota_np, "tqp": tqp})

    # ---- device program (cached on tiling geometry) ----
    key = (NT, T_b, NBLK, DIM)
    nc = _PROGRAM_CACHE.get(key)
    if nc is None:
        nc = _build_program(NT, T_b, NBLK, DIM)
        _PROGRAM_CACHE[key] = nc

    res = run_bass_kernel_spmd(nc, in_maps, core_ids=list(range(NCORES)))
    LAST_RESULTS = res

    # ---- host: inverse-permute per-core outputs into [n_tissue, DIM] ----
    yb = np.concatenate(
        [
            res.results[k]["y"].reshape(BLK, NBLK, DIM).transpose(1, 0, 2)
            for k in range(NCORES)
        ],
        axis=0,
    )  # [nblocks_g, BLK, DIM] in (block, localid) layout
    out = np.ascontiguousarray(yb[tissue2block, tissue2loc])
    return out
